# revision 33
# baseline (speedup 1.0000x reference)
"""Grouped single-step GRU (B=1024, U=8, I=H=512) on 8 trn2 NeuronCores.

Strategy: expert-parallel — core u computes GRU unit u for the whole batch.
Host pre-transposes operands so the device kernel is pure GEMM + fused
epilogue with zero on-chip transposes.

Precision split (rel-err budget is 2e-2; host-sim predicts 1.62e-2):
  - z-gate and n-gate-x-side matmuls run in bf16: full PE rate, FWL hides
    LDWEIGHTS. (fp8 on z or xn blows the error budget — verified by
    bit-exact host simulation: z-any-side 1.96e-2, xn 2.08e-2.)
  - r-gate (both sides) and n-gate-h-side matmuls run in fp8e4m3 with
    DoubleRow perf mode: the PE virtualizes to 128x256 so each matmul
    contracts 256 rows -> half the matmul count/time. Weights are
    host-scaled by 16 (into the fp8 normal range); the sigmoid/tanh
    activations un-scale via their scale operand (wn is host-scaled by
    16 in bf16 — exact — so pxn + r*phn share one scale).
  - epilogue keeps PSUM-fed ops in f32 and the gate combine in bf16
    (DVE 2x path); output stored bf16, host upcasts.

Scheduling (v2): the PE clock is HAM-gated — it starts at half rate,
reaches full rate only after ~5.4us of sustained matmul activity, and
drops back to half rate for ~6us if the matmul stream ever idles >2us.
So the whole kernel is built around a gapless matmul stream:
  - ALL loads are issued up front across both HWDGE DMA queues (sync +
    scalar) in consumption-deadline order; SBUF holds every tensor
    simultaneously (~67KB/partition of 208KB). (gpsimd SWDGE DMAs are
    not serviced by this runtime — verified: tensors arrive as garbage.)
  - bf16 weight slabs are batched into one dram tensor and fetched in 5
    grouped DMAs (fewer 600-800ns descriptor-issue ops).
  - the Scalar engine issues only 5 small DMAs then is dedicated to
    epilogue activations, so PSUM consumers never queue behind loads.
  - a short junk-matmul burst (4) bridges the ~1.5us until the first fp8
    operands land and starts the HAM warmup clock.
  - the epilogue keeps exactly 3 Scalar activations per block (r, z,
    tanh): a 4th pushes the Scalar act chain past the per-block matmul
    budget and the PE starts waiting on PSUM release.
  - the drain block's z/hn matmul groups and epilogue run in column
    halves so the last store chain starts half a block earlier.
"""

import sys

if "/opt/trn_rl_repo" not in sys.path:
    sys.path.insert(0, "/opt/trn_rl_repo")

from contextlib import ExitStack

import ml_dtypes
import numpy as np

import concourse.tile as tile
from concourse import bacc, mybir
from concourse.bass_utils import run_bass_kernel_spmd

B, U, I, H = 1024, 8, 512, 512
G = 3 * H
NB = 512          # moving-operand width (b-half)
NBH = B // NB     # 2 b-halves
KT = I // 128     # 4 contraction chunks
JT = H // 128     # 4 output-gate partition chunks
N_WARMUP_MM = 10  # bridge the first-load latency; HAM warms during it
W8SCALE = 16.0    # fp8 weight pre-scale (undone by the sigmoid scale)

F32 = mybir.dt.float32
BF16 = mybir.dt.bfloat16
F8 = mybir.dt.float8e4
AF = mybir.ActivationFunctionType
ALU = mybir.AluOpType
DR = mybir.MatmulPerfMode.DoubleRow
BF16NP = ml_dtypes.bfloat16
F8NP = ml_dtypes.float8_e4m3

LAST_EXEC_NS = None
TRACE = False
TRACE_DIR = None

_compiled = None


def _ensure_ntff_hook():
    """Provide antenv.axon_hooks + a ctypes NTFF hook when the image lacks
    them (mirrors trn_agent_boot's degraded-silently path), and keep trace
    artifacts local instead of uploading."""
    import contextlib
    import ctypes
    import types

    from concourse import bass_utils as _bu

    _bu.upload_artifacts = lambda tmpdir: f"local://{tmpdir}"

    try:
        from antenv.axon_hooks import get_axon_ntff_profile_hook  # noqa: F401

        return
    except ImportError:
        pass

    import antenv

    mod = types.ModuleType("antenv.axon_hooks")
    _holder = [None]
    mod.set_axon_ntff_profile_hook = lambda h: _holder.__setitem__(0, h)
    mod.get_axon_ntff_profile_hook = lambda: _holder[0]
    sys.modules["antenv.axon_hooks"] = mod
    antenv.axon_hooks = mod

    lib = ctypes.CDLL("/opt/axon/libaxon_pjrt.so")
    if not hasattr(lib, "axon_start_nrt_profile"):
        return
    lib.axon_start_nrt_profile.argtypes = [
        ctypes.POINTER(ctypes.c_int64),
        ctypes.c_size_t,
    ]
    lib.axon_start_nrt_profile.restype = ctypes.c_int64
    lib.axon_stop_nrt_profile.argtypes = [ctypes.c_char_p]
    lib.axon_stop_nrt_profile.restype = ctypes.c_int64

    @contextlib.contextmanager
    def _hook(output_dir, device_ids):
        import jax

        jax.devices()
        if device_ids:
            ids = (ctypes.c_int64 * len(device_ids))(*device_ids)
            rc = lib.axon_start_nrt_profile(ids, len(device_ids))
        else:
            rc = lib.axon_start_nrt_profile(None, 0)
        if rc != 0:
            raise RuntimeError(f"axon_start_nrt_profile rc={rc}")
        try:
            yield
        finally:
            n = lib.axon_stop_nrt_profile(str(output_dir).encode())
            print(f"ntff profile: {n} file(s) written to {output_dir}")

    mod.set_axon_ntff_profile_hook(_hook)


# bf16 weight slab order inside the batched `wall` dram tensor (each slab
# [128, 512]), grouped into 2-slab DMA batches ordered by consumption
# deadline: wn pairs early (block starts), wz/uz pairs at flush time.
# (un is fp8 — see un8.)
WALL_SLABS = [
    "wn0", "wn1", "wz0", "uz0", "wn2", "wn3",
    "wz1", "uz1", "wz2", "uz2", "wz3", "uz3",
]
WALL_COLS = 512 * len(WALL_SLABS)  # 6144


def _build():
    nc = bacc.Bacc(
        "TRN2",
        target_bir_lowering=False,
        debug=False,
        num_devices=U,
    )
    xT = nc.dram_tensor("xT", [NBH, 128, KT * NB], BF16, kind="ExternalInput").ap()
    hT = nc.dram_tensor("hT", [NBH, 128, KT * NB], BF16, kind="ExternalInput").ap()
    x8d = nc.dram_tensor("x8", [NBH, 128, KT, NB], F8, kind="ExternalInput").ap()
    h8d = nc.dram_tensor("h8", [NBH, 128, KT, NB], F8, kind="ExternalInput").ap()
    # all bf16 per-gate weight slabs batched: columns per WALL_SLABS, each
    # slab [j, p, k*128 + c] flattened to 512 columns
    wall = nc.dram_tensor("wall", [128, WALL_COLS], BF16, kind="ExternalInput").ap()
    # r-gate fp8 slabs, j-pairs batched for 2KB DMA lines:
    # [jp, p, jj, slot=(src*4+kk*2+ko), m]
    wr8d = nc.dram_tensor("wr8", [2, 128, 2, 8, 128], F8, kind="ExternalInput").ap()
    # n-gate h-side fp8 slabs (one 2KB-line DMA): [p, j, slot=(kk*2+ko), m]
    un8d = nc.dram_tensor("un8", [128, 4, 4, 128], F8, kind="ExternalInput").ap()
    biases = nc.dram_tensor("biases", [128, 20], F32, kind="ExternalInput").ap()
    outT = nc.dram_tensor("outT", [H, B], BF16, kind="ExternalOutput").ap()

    with tile.TileContext(nc) as tc, ExitStack() as ctx:
        wpool = ctx.enter_context(tc.tile_pool(name="w", bufs=1))
        xpool = ctx.enter_context(tc.tile_pool(name="x", bufs=1))
        bpool = ctx.enter_context(tc.tile_pool(name="b", bufs=1))
        # per-gate PSUM pools; 4+2+1+1 tiles of [128,512]f32 = all 8 banks.
        # The single-buf pools work because z/hn groups are deferred one
        # block and the Scalar engine (freed of DMA issue work) consumes
        # pz promptly.
        pp_r = ctx.enter_context(tc.tile_pool(name="ps_r", bufs=4, space="PSUM"))
        pp_xn = ctx.enter_context(tc.tile_pool(name="ps_xn", bufs=2, space="PSUM"))
        pp_z = ctx.enter_context(tc.tile_pool(name="ps_z", bufs=1, space="PSUM"))
        pp_hn = ctx.enter_context(tc.tile_pool(name="ps_hn", bufs=1, space="PSUM"))
        epool = ctx.enter_context(tc.tile_pool(name="work", bufs=3))
        # output tiles get their own deep pool: stores sit behind loads in
        # the sync DMA queue (FIFO), so completions are late — one tile per
        # block means no DVE op ever waits on a store.
        opool = ctx.enter_context(tc.tile_pool(name="out", bufs=8))

        # ---- SBUF tiles (everything lives simultaneously) ----
        jnk = bpool.tile([128, NB], BF16, tag="jnk")
        bt = bpool.tile([128, 20], F32, tag="bias")
        gt = [
            wpool.tile([128, 1024], BF16, tag=f"g{b}", name=f"g{b}")
            for b in range(6)
        ]
        wr8p = [
            wpool.tile([128, 2, 8, 128], F8, tag=f"wr8p{h}", name=f"wr8p{h}")
            for h in range(2)
        ]
        wr8_s = {j: wr8p[j // 2][:, j % 2] for j in range(JT)}
        un8t = wpool.tile([128, 4, 4, 128], F8, tag="un8")
        un8_s = {j: un8t[:, j] for j in range(JT)}
        x_s = {
            bh: xpool.tile([128, KT * NB], BF16, tag=f"x_{bh}", name=f"x_{bh}")
            for bh in range(NBH)
        }
        h_s = {
            bh: xpool.tile([128, KT * NB], BF16, tag=f"h_{bh}", name=f"h_{bh}")
            for bh in range(NBH)
        }
        x8_s = {
            bh: xpool.tile([128, KT, NB], F8, tag=f"x8_{bh}", name=f"x8_{bh}")
            for bh in range(NBH)
        }
        h8_s = {
            bh: xpool.tile([128, KT, NB], F8, tag=f"h8_{bh}", name=f"h8_{bh}")
            for bh in range(NBH)
        }

        # bf16 weight views: (name, j) -> [128, 512] slab slice, matching
        # WALL_SLABS batch order (2 slabs per gt tile)
        wg = {}
        for i, nm in enumerate(WALL_SLABS):
            c0 = (i % 2) * 512
            wg[nm[:2], int(nm[2])] = gt[i // 2][:, c0 : c0 + 512]

        # ---- upfront DMA issue, per queue in consumption-deadline order ----
        # (only the two HWDGE queues — sync and scalar — work in this
        # runtime; gpsimd SWDGE DMAs are not serviced.) The head is
        # aggregate-bandwidth-bound (~300B/ns for both queues), so bytes
        # are strictly deadline-ordered across the two queues.
        nc.gpsimd.memset(jnk[:], 0.0)

        # scalar HWDGE (engine must be free for epilogue activations from
        # ~16us, so all issues happen in the first ~6us): fp8 r weights
        # (j-paired for 2KB DMA lines), then the bf16 weight batches and
        # fp8 un slabs in flush order.
        nc.scalar.dma_start(out=wr8p[0][:], in_=wr8d[0])
        nc.scalar.dma_start(out=wr8p[1][:], in_=wr8d[1])
        nc.scalar.dma_start(out=gt[0][:], in_=wall[:, 0:1024])       # wn0 wn1
        nc.scalar.dma_start(out=un8t[:], in_=un8d)
        nc.scalar.dma_start(out=gt[1][:], in_=wall[:, 1024:2048])    # wz0 uz0
        nc.scalar.dma_start(out=gt[2][:], in_=wall[:, 2048:3072])    # wn2 wn3
        nc.scalar.dma_start(out=gt[3][:], in_=wall[:, 3072:4096])    # wz1 uz1
        nc.scalar.dma_start(out=gt[4][:], in_=wall[:, 4096:5120])    # wz2 uz2
        nc.scalar.dma_start(out=gt[5][:], in_=wall[:, 5120:6144])    # wz3 uz3
        nc.scalar.dma_start(out=bt[:], in_=biases[:])

        # sync HWDGE: the x/h stream — bh=0 fp8 first (r sweep), then bf16
        # x0/h0 split in half for early semaphores, then bh=1. Stores
        # queue behind these (FIFO) but nothing waits on store completion
        # until the drain.
        half = KT * NB // 2
        nc.sync.dma_start(out=x8_s[0][:], in_=x8d[0])
        nc.sync.dma_start(out=h8_s[0][:], in_=h8d[0])
        nc.sync.dma_start(out=x_s[0][:, :half], in_=xT[0][:, :half])
        nc.sync.dma_start(out=x_s[0][:, half:], in_=xT[0][:, half:])
        nc.sync.dma_start(out=h_s[0][:, :half], in_=hT[0][:, :half])
        nc.sync.dma_start(out=h_s[0][:, half:], in_=hT[0][:, half:])
        nc.sync.dma_start(out=x8_s[1][:], in_=x8d[1])
        nc.sync.dma_start(out=h8_s[1][:], in_=h8d[1])
        nc.sync.dma_start(out=x_s[1][:], in_=xT[1])
        nc.sync.dma_start(out=h_s[1][:], in_=hT[1])

        # ---- warmup junk matmuls (start HAM clock ramp, bridge loads) ----
        pjnk = pp_xn.tile([128, NB], F32, tag="pxn")
        for _ in range(N_WARMUP_MM):
            nc.tensor.matmul(
                pjnk[:], lhsT=jnk[:, 0:128], rhs=jnk[:], start=True, stop=True
            )

        def mm_group(pt, ops, stt=True, stp=True):
            for i, (w, r, pm) in enumerate(ops):
                nc.tensor.matmul(
                    pt,
                    lhsT=w,
                    rhs=r,
                    start=(i == 0 and stt),
                    stop=(i == len(ops) - 1 and stp),
                    perf_mode=pm,
                )

        def bf_ops(nm, bh, j, m_s, c0=0, c1=NB):
            return [
                (wg[nm, j][:, k * 128 : (k + 1) * 128],
                 m_s[:, k * NB + c0 : k * NB + c1], None)
                for k in range(KT)
            ]

        def z_ops(bh, j, c0=0, c1=NB):
            return bf_ops("wz", bh, j, x_s[bh], c0, c1) + bf_ops(
                "uz", bh, j, h_s[bh], c0, c1
            )

        def r_ops(bh, j, srcs=(0, 1)):
            ops = []
            for src in srcs:
                m8 = (x8_s, h8_s)[src][bh]
                for kk in range(2):
                    s0 = src * 4 + kk * 2
                    ops.append(
                        (wr8_s[j][:, s0 : s0 + 2, :],
                         m8[:, 2 * kk : 2 * kk + 2, :], DR)
                    )
            return ops

        def hn_ops(bh, j, c0=0, c1=NB):
            return [
                (un8_s[j][:, 2 * kk : 2 * kk + 2, :],
                 h8_s[bh][:, 2 * kk : 2 * kk + 2, c0:c1], DR)
                for kk in range(2)
            ]

        state = {}

        def make_epilogue(bh, j, pr, pz, pxn, phn):
            r_t = epool.tile([128, NB], F32, tag="r")
            z_t = epool.tile([128, NB], BF16, tag="z")
            t_t = epool.tile([128, NB], F32, tag="t")
            s_t = epool.tile([128, NB], F32, tag="s")
            n_t = epool.tile([128, NB], BF16, tag="n")
            d_t = epool.tile([128, NB], BF16, tag="d")
            e_t = epool.tile([128, NB], BF16, tag="e")
            o_t = opool.tile([128, NB], BF16, tag="o")

            def epilogue(c0, c1, z_late=False, store_eng=None):
                cs = slice(c0, c1)
                h_j = h_s[bh][:, j * NB : (j + 1) * NB]
                nc.scalar.activation(
                    r_t[:, cs], pr[:, cs], AF.Sigmoid,
                    bias=bt[:, j : j + 1], scale=1.0 / W8SCALE,
                )

                def act_z():
                    nc.scalar.activation(
                        z_t[:, cs], pz[:, cs], AF.Sigmoid,
                        bias=bt[:, 4 + j : 5 + j],
                    )

                if not z_late:
                    act_z()
                # t = (hn + b_hn) * r
                nc.vector.scalar_tensor_tensor(
                    t_t[:, cs], phn[:, cs], bt[:, 12 + j : 13 + j], r_t[:, cs],
                    op0=ALU.add, op1=ALU.mult,
                )
                nc.vector.tensor_tensor(
                    s_t[:, cs], t_t[:, cs], pxn[:, cs], op=ALU.add
                )
                # s is 16x-scaled (wn host-scaled, un8 fp8-scaled); the
                # tanh activation's scale operand unwinds it
                nc.scalar.activation(
                    n_t[:, cs], s_t[:, cs], AF.Tanh,
                    bias=bt[:, 8 + j : 9 + j], scale=1.0 / W8SCALE,
                )
                if z_late:
                    act_z()
                # out = n + z * (h - n), all-bf16 on the DVE 2x path
                nc.vector.tensor_tensor(
                    d_t[:, cs], h_j[:, cs], n_t[:, cs], op=ALU.subtract
                )
                nc.vector.tensor_tensor(
                    e_t[:, cs], z_t[:, cs], d_t[:, cs], op=ALU.mult
                )
                nc.vector.tensor_tensor(
                    o_t[:, cs], n_t[:, cs], e_t[:, cs], op=ALU.add
                )
                (store_eng or nc.sync).dma_start(
                    out=outT[
                        j * 128 : (j + 1) * 128, bh * NB + c0 : bh * NB + c1
                    ],
                    in_=o_t[:, cs],
                )

            return epilogue

        def flush_prev(prev):
            # hn (fp8 DR, 2 matmuls) first so the epilogue's STT can start
            # while the 8 z matmuls stream
            if prev is None:
                return
            pbh, pj = prev
            ps = state[prev]
            mm_group(ps["phn"][:], hn_ops(pbh, pj))
            mm_group(ps["pz"][:], z_ops(pbh, pj))
            ps["epilogue"](0, NB)

        prev = None
        prs = {}
        for bh in range(NBH):
            # fp8 r-gate sweep for the whole batch half: all x-side groups
            # first (they only need wr8 + x8), h-sides after
            prs[bh, 0] = pp_r.tile([128, NB], F32, tag="pr", name="pr")
            mm_group(prs[bh, 0][:], r_ops(bh, 0, srcs=(0,)), stp=False)
            flush_prev(prev)
            prev = None
            for j in range(1, JT):
                prs[bh, j] = pp_r.tile([128, NB], F32, tag="pr", name="pr")
                mm_group(prs[bh, j][:], r_ops(bh, j, srcs=(0,)), stp=False)
            for j in range(JT):
                mm_group(prs[bh, j][:], r_ops(bh, j, srcs=(1,)), stt=False)
            for j in range(JT):
                pxn = pp_xn.tile([128, NB], F32, tag="pxn")
                pz = pp_z.tile([128, NB], F32, tag="pz")
                phn = pp_hn.tile([128, NB], F32, tag="phn")
                mm_group(pxn[:], bf_ops("wn", bh, j, x_s[bh]))
                flush_prev(prev)
                state[bh, j] = {
                    "pz": pz,
                    "phn": phn,
                    "epilogue": make_epilogue(
                        bh, j, prs[bh, j], pz, pxn, phn
                    ),
                }
                prev = (bh, j)

        # drain block in column halves: each half's hn+z matmuls then its
        # epilogue, so the final store chain starts half a block earlier;
        # stores spread over both queues
        ps = state[prev]
        bh, j = prev
        hb = NB // 2
        for ci, store_eng in ((0, nc.scalar), (1, nc.sync)):
            c0, c1 = ci * hb, (ci + 1) * hb
            mm_group(ps["phn"][:, c0:c1], hn_ops(bh, j, c0, c1))
            mm_group(ps["pz"][:, c0:c1], z_ops(bh, j, c0, c1))
            ps["epilogue"](c0, c1, z_late=True, store_eng=store_eng)

    nc.compile()
    return nc


def _get_nc():
    global _compiled
    if _compiled is None:
        _compiled = _build()
    return _compiled


def _prep_in_maps(inputs, hidden, W_ih, W_hh, b_ih, b_hh):
    def pack_xh(a, np_dt):
        # [B, U, I] -> [U, bh, p, k*NB + b]: tile[p, k*NB+b] = a[bh*NB+b, u, k*128+p]
        a = np.asarray(a, dtype=np.float32)
        a5 = a.reshape(NBH, NB, U, KT, 128)  # [bh, b, u, k, p]
        return (
            a5.transpose(2, 0, 4, 3, 1).astype(np_dt).reshape(U, NBH, 128, KT * NB)
        )

    x = pack_xh(inputs, BF16NP)
    h = pack_xh(hidden, BF16NP)
    x8 = pack_xh(inputs, F8NP).reshape(U, NBH, 128, KT, NB)
    h8 = pack_xh(hidden, F8NP).reshape(U, NBH, 128, KT, NB)

    def pack_gate(W, g, scale=1.0):
        # gate-g rows -> [U, JT, 128, KT*128]: slab[j, p, k*128+c]
        # = scale * W[g*512 + j*128 + c, k*128 + p]
        wT = (
            np.asarray(W, dtype=np.float32)[:, g * H : (g + 1) * H, :]
            .transpose(0, 2, 1)
        ) * scale
        w5 = wT.reshape(U, KT, 128, JT, 128)  # [u, k, p, j, c]
        return (
            w5.transpose(0, 3, 2, 1, 4).astype(BF16NP).reshape(U, JT, 128, KT * 128)
        )

    # wn is host-scaled by 16 (exact in bf16) so pxn matches the 16x-scaled
    # fp8 hn accumulation; the tanh activation divides the sum back down.
    wn = pack_gate(W_ih, 2, W8SCALE)
    wz = pack_gate(W_ih, 1)
    uz = pack_gate(W_hh, 1)

    slab = {}
    for j in range(JT):
        slab[f"wn{j}"] = wn[:, j]
        slab[f"wz{j}"] = wz[:, j]
        slab[f"uz{j}"] = uz[:, j]
    wall = np.concatenate([slab[nm] for nm in WALL_SLABS], axis=2)  # [U,128,6144]

    def pack_r8(W):
        # r rows, fp8, pre-scaled: [u, j, p, kk, ko, m]
        w = np.asarray(W, dtype=np.float32)[:, :H, :] * W8SCALE
        w6 = w.reshape(U, JT, 128, KT // 2, 2, 128)  # [u,j,m,kk,ko,p]
        return w6.transpose(0, 1, 5, 3, 4, 2)  # [u,j,p,kk,ko,m]

    wr8 = (
        np.stack([pack_r8(W_ih), pack_r8(W_hh)], axis=3)  # [u,j,p,src,kk,ko,m]
        .astype(F8NP)
        .reshape(U, 2, 2, 128, 8, 128)  # [u, jp, jj, p, slot, m]
        .transpose(0, 1, 3, 2, 4, 5)    # [u, jp, p, jj, slot, m]
        .reshape(U, 2, 128, 2, 8, 128)
    )

    # n-gate h-side, fp8, pre-scaled by 16: [u, p, j, slot=(kk*2+ko), m]
    wun = np.asarray(W_hh, dtype=np.float32)[:, 2 * H :, :] * W8SCALE
    un8 = (
        wun.reshape(U, JT, 128, KT // 2, 2, 128)  # [u,j,m,kk,ko,p]
        .transpose(0, 5, 1, 3, 4, 2)              # [u,p,j,kk,ko,m]
        .astype(F8NP)
        .reshape(U, 128, 4, 4, 128)
    )

    bi = np.asarray(b_ih, dtype=np.float32)
    bhh = np.asarray(b_hh, dtype=np.float32)
    brz = bi[:, : 2 * H] + bhh[:, : 2 * H]  # r and z biases combine
    b_in = bi[:, 2 * H :]
    b_hn = bhh[:, 2 * H :] * W8SCALE  # added to the 16x-scaled phn by STT
    in_maps = []
    for u in range(U):
        # [128, 20] tile: column cls*4 + j holds bias_cls[j*128 + p]
        bb = np.stack(
            [brz[u, :H], brz[u, H:], b_in[u], b_hn[u], 0 * b_in[u]], axis=0
        )
        bb = bb.reshape(5, 4, 128).transpose(2, 0, 1).reshape(128, 20)
        in_maps.append(
            {
                "xT": x[u],
                "hT": h[u],
                "x8": x8[u],
                "h8": h8[u],
                "wall": np.ascontiguousarray(wall[u]),
                "wr8": wr8[u],
                "un8": un8[u],
                "biases": np.ascontiguousarray(bb),
            }
        )
    return in_maps


def kernel(inputs, hidden, W_ih, W_hh, b_ih, b_hh):
    global LAST_EXEC_NS
    nc = _get_nc()
    in_maps = _prep_in_maps(inputs, hidden, W_ih, W_hh, b_ih, b_hh)
    kwargs = {}
    if TRACE:
        _ensure_ntff_hook()
        if TRACE_DIR is not None:
            import os

            os.makedirs(TRACE_DIR, exist_ok=True)
            kwargs["tmpdir"] = TRACE_DIR
    res = run_bass_kernel_spmd(nc, in_maps, list(range(U)), trace=TRACE, **kwargs)
    LAST_EXEC_NS = res.exec_time_ns
    out = np.empty((B, U, H), dtype=np.float32)
    for u in range(U):
        out[:, u, :] = np.asarray(res.results[u]["outT"]).astype(np.float32).T
    return out


# revision 36
# speedup vs baseline: 1.1415x; 1.1415x over previous
"""Grouped single-step GRU (B=1024, U=8, I=H=512) on 8 trn2 NeuronCores.

Strategy: expert-parallel — core u computes GRU unit u for the whole batch.
Host pre-transposes operands so the device kernel is pure GEMM + fused
epilogue with zero on-chip transposes.

Precision split (rel-err budget is 2e-2; host-sim predicts 1.62e-2):
  - z-gate and n-gate-x-side matmuls run in bf16: full PE rate, FWL hides
    LDWEIGHTS. (fp8 on z or xn blows the error budget — verified by
    bit-exact host simulation: z-any-side 1.96e-2, xn 2.08e-2.)
  - r-gate (both sides) and n-gate-h-side matmuls run in fp8e4m3 with
    DoubleRow perf mode: the PE virtualizes to 128x256 so each matmul
    contracts 256 rows -> half the matmul count/time. Weights are
    host-scaled by 16 (into the fp8 normal range); the sigmoid/tanh
    activations un-scale via their scale operand (wn is host-scaled by
    16 in bf16 — exact — so pxn + r*phn share one scale).
  - epilogue keeps PSUM-fed ops in f32 and the gate combine in bf16
    (DVE 2x path); output stored bf16, host upcasts.

Scheduling (v2): the PE clock is HAM-gated — it starts at half rate,
reaches full rate only after ~5.4us of sustained matmul activity, and
drops back to half rate for ~6us if the matmul stream ever idles >2us.
So the whole kernel is built around a gapless matmul stream:
  - ALL loads are issued up front across both HWDGE DMA queues (sync +
    scalar) in consumption-deadline order; SBUF holds every tensor
    simultaneously (~67KB/partition of 208KB). (gpsimd SWDGE DMAs are
    not serviced by this runtime — verified: tensors arrive as garbage.)
  - bf16 weight slabs are batched into one dram tensor and fetched in 5
    grouped DMAs (fewer 600-800ns descriptor-issue ops).
  - the Scalar engine issues only 5 small DMAs then is dedicated to
    epilogue activations, so PSUM consumers never queue behind loads.
  - a short junk-matmul burst (4) bridges the ~1.5us until the first fp8
    operands land and starts the HAM warmup clock.
  - the epilogue keeps exactly 3 Scalar activations per block (r, z,
    tanh): a 4th pushes the Scalar act chain past the per-block matmul
    budget and the PE starts waiting on PSUM release.
  - the drain block's z/hn matmul groups and epilogue run in column
    halves so the last store chain starts half a block earlier.
"""

import sys

if "/opt/trn_rl_repo" not in sys.path:
    sys.path.insert(0, "/opt/trn_rl_repo")

from contextlib import ExitStack

import ml_dtypes
import numpy as np

import concourse.tile as tile
from concourse import bacc, mybir
from concourse.bass_utils import run_bass_kernel_spmd

B, U, I, H = 1024, 8, 512, 512
G = 3 * H
NB = 512          # moving-operand width (b-half)
NBH = B // NB     # 2 b-halves
KT = I // 128     # 4 contraction chunks
JT = H // 128     # 4 output-gate partition chunks
N_WARMUP_MM = 7   # bridge the first-load latency; HAM warms during it
W8SCALE = 16.0    # fp8 weight pre-scale (undone by the sigmoid scale)

F32 = mybir.dt.float32
BF16 = mybir.dt.bfloat16
F8 = mybir.dt.float8e4
AF = mybir.ActivationFunctionType
ALU = mybir.AluOpType
DR = mybir.MatmulPerfMode.DoubleRow
BF16NP = ml_dtypes.bfloat16
F8NP = ml_dtypes.float8_e4m3

LAST_EXEC_NS = None
TRACE = False
TRACE_DIR = None

_compiled = None


def _ensure_ntff_hook():
    """Provide antenv.axon_hooks + a ctypes NTFF hook when the image lacks
    them (mirrors trn_agent_boot's degraded-silently path), and keep trace
    artifacts local instead of uploading."""
    import contextlib
    import ctypes
    import types

    from concourse import bass_utils as _bu

    _bu.upload_artifacts = lambda tmpdir: f"local://{tmpdir}"

    try:
        from antenv.axon_hooks import get_axon_ntff_profile_hook  # noqa: F401

        return
    except ImportError:
        pass

    import antenv

    mod = types.ModuleType("antenv.axon_hooks")
    _holder = [None]
    mod.set_axon_ntff_profile_hook = lambda h: _holder.__setitem__(0, h)
    mod.get_axon_ntff_profile_hook = lambda: _holder[0]
    sys.modules["antenv.axon_hooks"] = mod
    antenv.axon_hooks = mod

    lib = ctypes.CDLL("/opt/axon/libaxon_pjrt.so")
    if not hasattr(lib, "axon_start_nrt_profile"):
        return
    lib.axon_start_nrt_profile.argtypes = [
        ctypes.POINTER(ctypes.c_int64),
        ctypes.c_size_t,
    ]
    lib.axon_start_nrt_profile.restype = ctypes.c_int64
    lib.axon_stop_nrt_profile.argtypes = [ctypes.c_char_p]
    lib.axon_stop_nrt_profile.restype = ctypes.c_int64

    @contextlib.contextmanager
    def _hook(output_dir, device_ids):
        import jax

        jax.devices()
        if device_ids:
            ids = (ctypes.c_int64 * len(device_ids))(*device_ids)
            rc = lib.axon_start_nrt_profile(ids, len(device_ids))
        else:
            rc = lib.axon_start_nrt_profile(None, 0)
        if rc != 0:
            raise RuntimeError(f"axon_start_nrt_profile rc={rc}")
        try:
            yield
        finally:
            n = lib.axon_stop_nrt_profile(str(output_dir).encode())
            print(f"ntff profile: {n} file(s) written to {output_dir}")

    mod.set_axon_ntff_profile_hook(_hook)


# bf16 weight slab order inside the batched `wall` dram tensor (each slab
# [128, 512]), grouped into 2-slab DMA batches ordered by consumption
# deadline: wn pairs early (block starts), wz/uz pairs at flush time.
# (un is fp8 — see un8.)
WALL_SLABS = [
    "wn0", "wn1", "wz0", "uz0", "wn2", "wn3",
    "wz1", "uz1", "wz2", "uz2", "wz3", "uz3",
]
WALL_COLS = 512 * len(WALL_SLABS)  # 6144


def _build():
    nc = bacc.Bacc(
        "TRN2",
        target_bir_lowering=False,
        debug=False,
        num_devices=U,
    )
    xT = nc.dram_tensor("xT", [NBH, 128, KT * NB], BF16, kind="ExternalInput").ap()
    hT = nc.dram_tensor("hT", [NBH, 128, KT * NB], BF16, kind="ExternalInput").ap()
    x8d = nc.dram_tensor("x8", [NBH, 128, KT, NB], F8, kind="ExternalInput").ap()
    h8d = nc.dram_tensor("h8", [NBH, 128, KT, NB], F8, kind="ExternalInput").ap()
    # all bf16 per-gate weight slabs batched: columns per WALL_SLABS, each
    # slab [j, p, k*128 + c] flattened to 512 columns
    wall = nc.dram_tensor("wall", [128, WALL_COLS], BF16, kind="ExternalInput").ap()
    # r-gate fp8 slabs, j-pairs batched for 2KB DMA lines:
    # [jp, p, jj, slot=(src*4+kk*2+ko), m]
    wr8d = nc.dram_tensor("wr8", [2, 128, 2, 8, 128], F8, kind="ExternalInput").ap()
    # n-gate h-side fp8 slabs (one 2KB-line DMA): [p, j, slot=(kk*2+ko), m]
    un8d = nc.dram_tensor("un8", [128, 4, 4, 128], F8, kind="ExternalInput").ap()
    biases = nc.dram_tensor("biases", [128, 20], F32, kind="ExternalInput").ap()
    outT = nc.dram_tensor("outT", [H, B], BF16, kind="ExternalOutput").ap()

    with tile.TileContext(nc) as tc, ExitStack() as ctx:
        wpool = ctx.enter_context(tc.tile_pool(name="w", bufs=1))
        xpool = ctx.enter_context(tc.tile_pool(name="x", bufs=1))
        bpool = ctx.enter_context(tc.tile_pool(name="b", bufs=1))
        # per-gate PSUM pools; 4+2+1+1 tiles of [128,512]f32 = all 8 banks.
        # The single-buf pools work because z/hn groups are deferred one
        # block and the Scalar engine (freed of DMA issue work) consumes
        # pz promptly.
        pp_r = ctx.enter_context(tc.tile_pool(name="ps_r", bufs=4, space="PSUM"))
        pp_xn = ctx.enter_context(tc.tile_pool(name="ps_xn", bufs=2, space="PSUM"))
        pp_z = ctx.enter_context(tc.tile_pool(name="ps_z", bufs=1, space="PSUM"))
        pp_hn = ctx.enter_context(tc.tile_pool(name="ps_hn", bufs=1, space="PSUM"))
        epool = ctx.enter_context(tc.tile_pool(name="work", bufs=3))
        # output tiles get their own deep pool: stores sit behind loads in
        # the sync DMA queue (FIFO), so completions are late — one tile per
        # block means no DVE op ever waits on a store.
        opool = ctx.enter_context(tc.tile_pool(name="out", bufs=8))

        # ---- SBUF tiles (everything lives simultaneously) ----
        jnk = bpool.tile([128, NB], BF16, tag="jnk")
        bt = bpool.tile([128, 20], F32, tag="bias")
        gt = [
            wpool.tile([128, 1024], BF16, tag=f"g{b}", name=f"g{b}")
            for b in range(6)
        ]
        wr8p = [
            wpool.tile([128, 2, 8, 128], F8, tag=f"wr8p{h}", name=f"wr8p{h}")
            for h in range(2)
        ]
        wr8_s = {j: wr8p[j // 2][:, j % 2] for j in range(JT)}
        un8t = wpool.tile([128, 4, 4, 128], F8, tag="un8")
        un8_s = {j: un8t[:, j] for j in range(JT)}
        x_s = {
            bh: xpool.tile([128, KT * NB], BF16, tag=f"x_{bh}", name=f"x_{bh}")
            for bh in range(NBH)
        }
        h_s = {
            bh: xpool.tile([128, KT * NB], BF16, tag=f"h_{bh}", name=f"h_{bh}")
            for bh in range(NBH)
        }
        x8_s = {
            bh: xpool.tile([128, KT, NB], F8, tag=f"x8_{bh}", name=f"x8_{bh}")
            for bh in range(NBH)
        }
        h8_s = {
            bh: xpool.tile([128, KT, NB], F8, tag=f"h8_{bh}", name=f"h8_{bh}")
            for bh in range(NBH)
        }

        # bf16 weight views: (name, j) -> [128, 512] slab slice, matching
        # WALL_SLABS batch order (2 slabs per gt tile)
        wg = {}
        for i, nm in enumerate(WALL_SLABS):
            c0 = (i % 2) * 512
            wg[nm[:2], int(nm[2])] = gt[i // 2][:, c0 : c0 + 512]

        # ---- upfront DMA issue, per queue in consumption-deadline order ----
        # (only the two HWDGE queues — sync and scalar — work in this
        # runtime; gpsimd SWDGE DMAs are not serviced.) The head is
        # aggregate-bandwidth-bound (~300B/ns for both queues), so bytes
        # are strictly deadline-ordered across the two queues.
        nc.gpsimd.memset(jnk[:], 0.0)

        # scalar HWDGE (engine must be free for epilogue activations from
        # ~16us, so all issues happen in the first ~6us): fp8 r weights
        # (j-paired for 2KB DMA lines), then the bf16 weight batches and
        # fp8 un slabs in flush order.
        nc.scalar.dma_start(out=bt[:], in_=biases[:])
        nc.scalar.dma_start(out=wr8p[0][:], in_=wr8d[0])
        nc.scalar.dma_start(out=wr8p[1][:], in_=wr8d[1])
        nc.scalar.dma_start(out=gt[0][:], in_=wall[:, 0:1024])       # wn0 wn1
        nc.scalar.dma_start(out=un8t[:], in_=un8d)
        nc.scalar.dma_start(out=gt[1][:], in_=wall[:, 1024:2048])    # wz0 uz0
        nc.scalar.dma_start(out=gt[2][:], in_=wall[:, 2048:3072])    # wn2 wn3
        nc.scalar.dma_start(out=gt[3][:], in_=wall[:, 3072:4096])    # wz1 uz1
        nc.scalar.dma_start(out=gt[4][:], in_=wall[:, 4096:5120])    # wz2 uz2
        nc.scalar.dma_start(out=gt[5][:], in_=wall[:, 5120:6144])    # wz3 uz3

        # sync HWDGE: the x/h stream — bh=0 fp8 first (r sweep), then bf16
        # x0/h0 split in half for early semaphores, then bh=1. Stores
        # queue behind these (FIFO) but nothing waits on store completion
        # until the drain.
        half = KT * NB // 2
        nc.sync.dma_start(out=x8_s[0][:], in_=x8d[0])
        nc.sync.dma_start(out=h8_s[0][:], in_=h8d[0])
        nc.sync.dma_start(out=x_s[0][:, :half], in_=xT[0][:, :half])
        nc.sync.dma_start(out=x_s[0][:, half:], in_=xT[0][:, half:])
        nc.sync.dma_start(out=h_s[0][:, :half], in_=hT[0][:, :half])
        nc.sync.dma_start(out=h_s[0][:, half:], in_=hT[0][:, half:])
        nc.sync.dma_start(out=x8_s[1][:], in_=x8d[1])
        nc.sync.dma_start(out=h8_s[1][:], in_=h8d[1])
        nc.sync.dma_start(out=x_s[1][:], in_=xT[1])
        nc.sync.dma_start(out=h_s[1][:], in_=hT[1])

        # ---- warmup junk matmuls (start HAM clock ramp, bridge loads) ----
        pjnk = pp_xn.tile([128, NB], F32, tag="pxn")
        for _ in range(N_WARMUP_MM):
            nc.tensor.matmul(
                pjnk[:], lhsT=jnk[:, 0:128], rhs=jnk[:], start=True, stop=True
            )

        def mm_group(pt, ops, stt=True, stp=True):
            for i, (w, r, pm) in enumerate(ops):
                nc.tensor.matmul(
                    pt,
                    lhsT=w,
                    rhs=r,
                    start=(i == 0 and stt),
                    stop=(i == len(ops) - 1 and stp),
                    perf_mode=pm,
                )

        def bf_ops(nm, bh, j, m_s, c0=0, c1=NB):
            return [
                (wg[nm, j][:, k * 128 : (k + 1) * 128],
                 m_s[:, k * NB + c0 : k * NB + c1], None)
                for k in range(KT)
            ]

        def z_ops(bh, j, c0=0, c1=NB):
            return bf_ops("wz", bh, j, x_s[bh], c0, c1) + bf_ops(
                "uz", bh, j, h_s[bh], c0, c1
            )

        def r_ops(bh, j, srcs=(0, 1)):
            ops = []
            for src in srcs:
                m8 = (x8_s, h8_s)[src][bh]
                for kk in range(2):
                    s0 = src * 4 + kk * 2
                    ops.append(
                        (wr8_s[j][:, s0 : s0 + 2, :],
                         m8[:, 2 * kk : 2 * kk + 2, :], DR)
                    )
            return ops

        def hn_ops(bh, j, c0=0, c1=NB):
            return [
                (un8_s[j][:, 2 * kk : 2 * kk + 2, :],
                 h8_s[bh][:, 2 * kk : 2 * kk + 2, c0:c1], DR)
                for kk in range(2)
            ]

        state = {}

        def make_epilogue(bh, j, pr, pz, pxn, phn):
            r_t = epool.tile([128, NB], F32, tag="r")
            z_t = epool.tile([128, NB], BF16, tag="z")
            t_t = epool.tile([128, NB], F32, tag="t")
            s_t = epool.tile([128, NB], F32, tag="s")
            n_t = epool.tile([128, NB], BF16, tag="n")
            d_t = epool.tile([128, NB], BF16, tag="d")
            e_t = epool.tile([128, NB], BF16, tag="e")
            o_t = opool.tile([128, NB], BF16, tag="o")

            def epilogue(c0, c1, z_late=False, store_eng=None):
                cs = slice(c0, c1)
                h_j = h_s[bh][:, j * NB : (j + 1) * NB]
                nc.scalar.activation(
                    r_t[:, cs], pr[:, cs], AF.Sigmoid,
                    bias=bt[:, j : j + 1], scale=1.0 / W8SCALE,
                )

                def act_z():
                    nc.scalar.activation(
                        z_t[:, cs], pz[:, cs], AF.Sigmoid,
                        bias=bt[:, 4 + j : 5 + j],
                    )

                if not z_late:
                    act_z()
                # t = (hn + b_hn) * r
                nc.vector.scalar_tensor_tensor(
                    t_t[:, cs], phn[:, cs], bt[:, 12 + j : 13 + j], r_t[:, cs],
                    op0=ALU.add, op1=ALU.mult,
                )
                nc.vector.tensor_tensor(
                    s_t[:, cs], t_t[:, cs], pxn[:, cs], op=ALU.add
                )
                # s is 16x-scaled (wn host-scaled, un8 fp8-scaled); the
                # tanh activation's scale operand unwinds it
                nc.scalar.activation(
                    n_t[:, cs], s_t[:, cs], AF.Tanh,
                    bias=bt[:, 8 + j : 9 + j], scale=1.0 / W8SCALE,
                )
                if z_late:
                    act_z()
                # out = n + z * (h - n), all-bf16 on the DVE 2x path
                nc.vector.tensor_tensor(
                    d_t[:, cs], h_j[:, cs], n_t[:, cs], op=ALU.subtract
                )
                nc.vector.tensor_tensor(
                    e_t[:, cs], z_t[:, cs], d_t[:, cs], op=ALU.mult
                )
                nc.vector.tensor_tensor(
                    o_t[:, cs], n_t[:, cs], e_t[:, cs], op=ALU.add
                )
                (store_eng or nc.sync).dma_start(
                    out=outT[
                        j * 128 : (j + 1) * 128, bh * NB + c0 : bh * NB + c1
                    ],
                    in_=o_t[:, cs],
                )

            return epilogue

        def flush_prev(prev):
            # hn (fp8 DR, 2 matmuls) first so the epilogue's STT can start
            # while the 8 z matmuls stream
            if prev is None:
                return
            pbh, pj = prev
            ps = state[prev]
            mm_group(ps["phn"][:], hn_ops(pbh, pj))
            mm_group(ps["pz"][:], z_ops(pbh, pj))
            ps["epilogue"](0, NB)

        prev = None
        prs = {}
        for bh in range(NBH):
            # fp8 r-gate sweep for the whole batch half: all x-side groups
            # first (they only need wr8 + x8), h-sides after
            prs[bh, 0] = pp_r.tile([128, NB], F32, tag="pr", name="pr")
            mm_group(prs[bh, 0][:], r_ops(bh, 0, srcs=(0,)), stp=False)
            flush_prev(prev)
            prev = None
            for j in range(1, JT):
                prs[bh, j] = pp_r.tile([128, NB], F32, tag="pr", name="pr")
                mm_group(prs[bh, j][:], r_ops(bh, j, srcs=(0,)), stp=False)
            for j in range(JT):
                mm_group(prs[bh, j][:], r_ops(bh, j, srcs=(1,)), stt=False)
            for j in range(JT):
                pxn = pp_xn.tile([128, NB], F32, tag="pxn")
                pz = pp_z.tile([128, NB], F32, tag="pz")
                phn = pp_hn.tile([128, NB], F32, tag="phn")
                mm_group(pxn[:], bf_ops("wn", bh, j, x_s[bh]))
                flush_prev(prev)
                state[bh, j] = {
                    "pz": pz,
                    "phn": phn,
                    "epilogue": make_epilogue(
                        bh, j, prs[bh, j], pz, pxn, phn
                    ),
                }
                prev = (bh, j)

        # drain block in column halves: each half's hn+z matmuls then its
        # epilogue, so the final store chain starts half a block earlier;
        # stores spread over both queues
        ps = state[prev]
        bh, j = prev
        hb = NB // 2
        for ci, store_eng in ((0, nc.scalar), (1, nc.sync)):
            c0, c1 = ci * hb, (ci + 1) * hb
            mm_group(ps["phn"][:, c0:c1], hn_ops(bh, j, c0, c1))
            mm_group(ps["pz"][:, c0:c1], z_ops(bh, j, c0, c1))
            ps["epilogue"](c0, c1, z_late=True, store_eng=store_eng)

    nc.compile()
    return nc


def _get_nc():
    global _compiled
    if _compiled is None:
        _compiled = _build()
    return _compiled


def _prep_in_maps(inputs, hidden, W_ih, W_hh, b_ih, b_hh):
    def pack_xh(a, np_dt):
        # [B, U, I] -> [U, bh, p, k*NB + b]: tile[p, k*NB+b] = a[bh*NB+b, u, k*128+p]
        a = np.asarray(a, dtype=np.float32)
        a5 = a.reshape(NBH, NB, U, KT, 128)  # [bh, b, u, k, p]
        return (
            a5.transpose(2, 0, 4, 3, 1).astype(np_dt).reshape(U, NBH, 128, KT * NB)
        )

    x = pack_xh(inputs, BF16NP)
    h = pack_xh(hidden, BF16NP)
    x8 = pack_xh(inputs, F8NP).reshape(U, NBH, 128, KT, NB)
    h8 = pack_xh(hidden, F8NP).reshape(U, NBH, 128, KT, NB)

    def pack_gate(W, g, scale=1.0):
        # gate-g rows -> [U, JT, 128, KT*128]: slab[j, p, k*128+c]
        # = scale * W[g*512 + j*128 + c, k*128 + p]
        wT = (
            np.asarray(W, dtype=np.float32)[:, g * H : (g + 1) * H, :]
            .transpose(0, 2, 1)
        ) * scale
        w5 = wT.reshape(U, KT, 128, JT, 128)  # [u, k, p, j, c]
        return (
            w5.transpose(0, 3, 2, 1, 4).astype(BF16NP).reshape(U, JT, 128, KT * 128)
        )

    # wn is host-scaled by 16 (exact in bf16) so pxn matches the 16x-scaled
    # fp8 hn accumulation; the tanh activation divides the sum back down.
    wn = pack_gate(W_ih, 2, W8SCALE)
    wz = pack_gate(W_ih, 1)
    uz = pack_gate(W_hh, 1)

    slab = {}
    for j in range(JT):
        slab[f"wn{j}"] = wn[:, j]
        slab[f"wz{j}"] = wz[:, j]
        slab[f"uz{j}"] = uz[:, j]
    wall = np.concatenate([slab[nm] for nm in WALL_SLABS], axis=2)  # [U,128,6144]

    def pack_r8(W):
        # r rows, fp8, pre-scaled: [u, j, p, kk, ko, m]
        w = np.asarray(W, dtype=np.float32)[:, :H, :] * W8SCALE
        w6 = w.reshape(U, JT, 128, KT // 2, 2, 128)  # [u,j,m,kk,ko,p]
        return w6.transpose(0, 1, 5, 3, 4, 2)  # [u,j,p,kk,ko,m]

    wr8 = (
        np.stack([pack_r8(W_ih), pack_r8(W_hh)], axis=3)  # [u,j,p,src,kk,ko,m]
        .astype(F8NP)
        .reshape(U, 2, 2, 128, 8, 128)  # [u, jp, jj, p, slot, m]
        .transpose(0, 1, 3, 2, 4, 5)    # [u, jp, p, jj, slot, m]
        .reshape(U, 2, 128, 2, 8, 128)
    )

    # n-gate h-side, fp8, pre-scaled by 16: [u, p, j, slot=(kk*2+ko), m]
    wun = np.asarray(W_hh, dtype=np.float32)[:, 2 * H :, :] * W8SCALE
    un8 = (
        wun.reshape(U, JT, 128, KT // 2, 2, 128)  # [u,j,m,kk,ko,p]
        .transpose(0, 5, 1, 3, 4, 2)              # [u,p,j,kk,ko,m]
        .astype(F8NP)
        .reshape(U, 128, 4, 4, 128)
    )

    bi = np.asarray(b_ih, dtype=np.float32)
    bhh = np.asarray(b_hh, dtype=np.float32)
    brz = bi[:, : 2 * H] + bhh[:, : 2 * H]  # r and z biases combine
    b_in = bi[:, 2 * H :]
    b_hn = bhh[:, 2 * H :] * W8SCALE  # added to the 16x-scaled phn by STT
    in_maps = []
    for u in range(U):
        # [128, 20] tile: column cls*4 + j holds bias_cls[j*128 + p]
        bb = np.stack(
            [brz[u, :H], brz[u, H:], b_in[u], b_hn[u], 0 * b_in[u]], axis=0
        )
        bb = bb.reshape(5, 4, 128).transpose(2, 0, 1).reshape(128, 20)
        in_maps.append(
            {
                "xT": x[u],
                "hT": h[u],
                "x8": x8[u],
                "h8": h8[u],
                "wall": np.ascontiguousarray(wall[u]),
                "wr8": wr8[u],
                "un8": un8[u],
                "biases": np.ascontiguousarray(bb),
            }
        )
    return in_maps


def kernel(inputs, hidden, W_ih, W_hh, b_ih, b_hh):
    global LAST_EXEC_NS
    nc = _get_nc()
    in_maps = _prep_in_maps(inputs, hidden, W_ih, W_hh, b_ih, b_hh)
    kwargs = {}
    if TRACE:
        _ensure_ntff_hook()
        if TRACE_DIR is not None:
            import os

            os.makedirs(TRACE_DIR, exist_ok=True)
            kwargs["tmpdir"] = TRACE_DIR
    res = run_bass_kernel_spmd(nc, in_maps, list(range(U)), trace=TRACE, **kwargs)
    LAST_EXEC_NS = res.exec_time_ns
    out = np.empty((B, U, H), dtype=np.float32)
    for u in range(U):
        out[:, u, :] = np.asarray(res.results[u]["outT"]).astype(np.float32).T
    return out
